# revision 66
# baseline (speedup 1.0000x reference)
# Trainium2 Bass kernel for nn_EquShiftQ2DF3P40 (group-equivariant CNN + dynamic filter).
#
# Sharding: batch 256 -> 32 samples/core on 8 cores. All weights replicated
# except the first es_fc layer (16384x1024), which is K-split across cores
# (each core contracts a 2048-feature slice for ALL 256 samples) followed by a
# ReduceScatter along the batch dim so each core ends with es1 pre-activations
# for exactly its own 32 samples.
#
# Compute layout notes:
# - convs run channels-on-partitions; 3x3 taps become accumulating matmuls with
#   shifted access patterns; conv2/ihc2 stack two dy taps on the partition axis
#   (a shifted copy of the input lives at the upper partitions) to raise K.
# - conv1/ihc1 have Cin=1, so a 9-partition im2col (9 shifted DMA copies of the
#   padded image) makes K=9.
# - the in-hand FC (6400->512) runs as 100 pixel-wise accumulating matmuls with
#   the per-pixel activation block as the stationary operand.
# - the dynamic-filter tail stays batch-major (samples on partitions) and runs
#   on the vector engine as broadcast-mul + segmented reduce.
import numpy as np
import ml_dtypes

import concourse.bacc as bacc
import concourse.mybir as mybir
from concourse.bass_utils import run_bass_kernel_spmd
from concourse import tile
import bass_rust

f32 = mybir.dt.float32
bf16 = mybir.dt.bfloat16
AF = mybir.ActivationFunctionType
ALU = mybir.AluOpType
bf = ml_dtypes.bfloat16

NCORES = 8
BC = 32  # samples per core
KSLICE = 16384 // NCORES  # es1 contraction slice per core


# ---------------------------------------------------------------- host prep
def _rot(x, g):
    return np.rot90(x, k=g, axes=(-2, -1))


def _sym(k):
    return 0.5 * (k + _rot(k, 2))


def _expand_tq(kappa):
    kappa = _sym(kappa)
    Co, Ci, kh, kw = kappa.shape
    W = np.stack([_rot(kappa, g) for g in range(2)], axis=1)
    return W.reshape(Co * 2, Ci, kh, kw)


def _expand_qq(kappa):
    kappa = _sym(kappa)
    Co, Ci, F, kh, kw = kappa.shape
    W = np.stack([_rot(np.roll(kappa, g, axis=2), g) for g in range(F)], axis=1)
    return W.reshape(Co * F, Ci * F, kh, kw)


def _pad_flat(x):
    # (B,1,40,40) -> (B, 42*42 + 96) zero-padded flat images (pad=1 border,
    # plus a tail pad so shifted contiguous im2col reads stay in bounds)
    B = x.shape[0]
    p = np.zeros((B, 42, 42), np.float32)
    p[:, 1:41, 1:41] = x[:, 0]
    out = np.zeros((B, 42 * 42 + 96), np.float32)
    out[:, : 42 * 42] = p.reshape(B, -1)
    return out


def _im2col9(flat):
    # (B, 1860) -> (B, 9, 1764): 9 (dy,dx)-shifted flat 42x42 images
    B = flat.shape[0]
    out = np.zeros((B, 9, 1764), np.float32)
    for dy in range(3):
        for dx in range(3):
            out[:, 3 * dy + dx] = flat[:, 42 * dy + dx:42 * dy + dx + 1764]
    return out


def host_prep(inputs):
    """Returns (shared_map, per_core_maps) of numpy arrays keyed by dram names."""
    obs = np.asarray(inputs["obs_encoding"], np.float32)
    patch = np.asarray(inputs["patch"], np.float32)
    B = obs.shape[0]

    W1e = _expand_tq(np.asarray(inputs["k1"], np.float32))        # (64,1,3,3)
    W2e = _expand_qq(np.asarray(inputs["k2"], np.float32))        # (128,64,3,3)
    W3e = _expand_qq(np.asarray(inputs["k3"], np.float32))        # (256,128,3,3)
    W4e = _expand_qq(np.asarray(inputs["k4"], np.float32))        # (64,256,3,3)
    W5e = _expand_qq(np.asarray(inputs["k5"], np.float32))        # (32,64,3,3)

    sh = {}
    # conv1 im2col weights, replicated at partition bases 0/32/64/96 for
    # 4-way row-tiled matmuls: (128, o) with rows 32s+q = w[q]
    w1f = W1e[:, 0].reshape(64, 9).T                      # (9, 64)
    wi1f = np.asarray(inputs["Wi1"], np.float32)[:, 0].reshape(32, 9).T  # (9, 32)
    w1r = np.zeros((128, 64), np.float32)
    wi1r = np.zeros((128, 32), np.float32)
    for s in range(4):
        w1r[32 * s:32 * s + 9] = w1f
        wi1r[32 * s:32 * s + 9] = wi1f
    sh["w1"] = w1r.astype(bf).copy()
    sh["wi1"] = wi1r.astype(bf).copy()
    # conv2 dy-stacked: w2a (3,128,128) rows=(dy0 ci | dy1 ci); w2b (3,64,128) dy2
    w2a = np.zeros((3, 128, 128), np.float32)
    w2b = np.zeros((3, 64, 128), np.float32)
    for dx in range(3):
        w2a[dx, 0:64] = W2e[:, :, 0, dx].T
        w2a[dx, 64:128] = W2e[:, :, 1, dx].T
        w2b[dx] = W2e[:, :, 2, dx].T
    sh["w2a"] = w2a.astype(bf)
    sh["w2ao"] = w2a[:, list(range(64, 128)) + list(range(64))].astype(bf).copy()
    sh["w2b"] = w2b.astype(bf)
    # conv3: (2,3,3,128,128) [mt][dy][dx][ci][o]
    w3 = np.zeros((2, 3, 3, 128, 128), np.float32)
    for mt in range(2):
        for dy in range(3):
            for dx in range(3):
                w3[mt, dy, dx] = W3e[mt * 128:(mt + 1) * 128, :, dy, dx].T
    sh["w3"] = w3.astype(bf)
    # conv4: (2,3,3,128,64) [kt][dy][dx][ci][o]
    w4 = np.zeros((2, 3, 3, 128, 64), np.float32)
    for kt in range(2):
        for dy in range(3):
            for dx in range(3):
                w4[kt, dy, dx] = W4e[:, kt * 128:(kt + 1) * 128, dy, dx].T
    sh["w4"] = w4.astype(bf)
    # conv5: (9,64,32)
    w5 = np.zeros((9, 64, 32), np.float32)
    for dy in range(3):
        for dx in range(3):
            w5[dy * 3 + dx] = W5e[:, :, dy, dx].T
    sh["w5"] = w5.astype(bf)
    # ihc2 dy-stacked (stride 2): wi2a (3,64,64) rows=(dy0 ci | dy1 ci); wi2b (3,32,64)
    Wi2 = np.asarray(inputs["Wi2"], np.float32)
    wi2a = np.zeros((3, 64, 64), np.float32)
    wi2b = np.zeros((3, 32, 64), np.float32)
    for dx in range(3):
        wi2a[dx, 0:32] = Wi2[:, :, 0, dx].T
        wi2a[dx, 32:64] = Wi2[:, :, 1, dx].T
        wi2b[dx] = Wi2[:, :, 2, dx].T
    sh["wi2a"] = wi2a.astype(bf)
    sh["wi2ao"] = wi2a[:, list(range(32, 64)) + list(range(32))].astype(bf).copy()
    sh["wi2b"] = wi2b.astype(bf)

    # --- consolidated weight stream: one (128, C) bf16 tensor, [p, c] ---
    # layout: w2a(3,128) w2ao(3,128) w2b2(3,128) w3(18,128) w4(18,64)
    #         wes2(8,512) wdf(8,528) wi2A2(3,64) wi2B2(3,64)
    w2b2 = np.concatenate([w2b, w2b], axis=1)           # (3,128,128) rows 0-63=dy2, 64-127=dy2
    # ihc2 weights on 128 partitions: E stack at rows 0:64, O at 64:128 (same
    # values — O parity is un-mirrored in the merged xih layout); dy2 at rows
    # 32:64 (E) / 96:128 (O).
    wi2A2 = np.tile(wi2a, (1, 2, 1))                     # (3,128,64)
    wi2B2 = np.zeros((3, 128, 64), np.float32)
    wi2B2[:, 32:64] = wi2b
    wi2B2[:, 96:128] = wi2b
    wbig = np.concatenate([
        w2a.transpose(1, 0, 2).reshape(128, 384),
        w2a[:, list(range(64, 128)) + list(range(64))].transpose(1, 0, 2).reshape(128, 384),
        w2b2.transpose(1, 0, 2).reshape(128, 384),
        np.ascontiguousarray(w3.transpose(3, 0, 1, 2, 4)).reshape(128, 2304),
        np.ascontiguousarray(w4.transpose(3, 0, 1, 2, 4)).reshape(128, 1152),
        np.asarray(inputs["Wes2"], np.float32).reshape(8, 128, 512).transpose(1, 0, 2).reshape(128, 4096),
        np.asarray(inputs["Wdf"], np.float32).reshape(8, 128, 528).transpose(1, 0, 2).reshape(128, 4224),
        wi2A2.transpose(1, 0, 2).reshape(128, 192),
        wi2B2.transpose(1, 0, 2).reshape(128, 192),
        np.tile(np.eye(32, dtype=np.float32), (4, 1)),   # sel: sums 4 col groups
    ], axis=1)
    sh["wbig"] = wbig.astype(bf).copy()                  # (128, 13344)
    # 64-partition weights: w5(9,32) + ident(32) at partitions 0-31
    w5s = np.zeros((9, 64, 32), np.float32)
    for dy in range(3):
        for dx in range(3):
            w5s[dy * 3 + dx] = W5e[:, :, dy, dx].T
    idpad = np.zeros((64, 32), np.float32)
    idpad[0:32] = np.eye(32, dtype=np.float32)
    wsml = np.concatenate([w5s.transpose(1, 0, 2).reshape(64, 288), idpad], axis=1)
    sh["wsml"] = wsml.astype(bf).copy()                  # (64, 320)
    # in-hand FC pix-paired: (128,50,512): rows 0-63 = (ch, pix q), 64-127 = (ch, pix q+50)
    wif3 = np.asarray(inputs["Wif"], np.float32).reshape(64, 100, 512)
    sh["wifp2"] = np.concatenate([wif3[:, :50], wif3[:, 50:]], axis=0).astype(bf).copy()

    # --- consolidated f32 constants: one (128, 1223) tensor ---
    b1e = np.repeat(np.asarray(inputs["b1"], np.float32), 2)
    b2e = np.repeat(np.asarray(inputs["b2"], np.float32), 2)
    b3e = np.repeat(np.asarray(inputs["b3"], np.float32), 2)
    b4e = np.repeat(np.asarray(inputs["b4"], np.float32), 2)
    b5e = np.repeat(np.asarray(inputs["b5"], np.float32), 2)
    kappa2 = np.asarray(inputs["kappa2"], np.float32)
    W2f = np.stack([np.roll(kappa2, g, axis=2) for g in range(2)], axis=1).reshape(4, 32)
    bcons = np.zeros((128, 1223), np.float32)
    bcons[:, 0] = np.concatenate([b1e, b1e])                                   # bc1
    bcons[:, 1] = np.tile(np.asarray(inputs["bi1"], np.float32), 4)            # bi1c
    bcons[:, 2] = b2e                                                          # bc2
    bcons[:, 3:5] = b3e.reshape(128, 2, order="F")                             # bc3
    bcons[0:64, 5] = b4e                                                       # bc4
    bcons[:, 6] = np.tile(np.asarray(inputs["bi2"], np.float32), 2)            # bi2c
    bcons[:, 7:15] = np.asarray(inputs["bes1"], np.float32).reshape(8, 128).T  # bes1t
    bcons[:, 15:19] = np.asarray(inputs["bes2"], np.float32).reshape(4, 128).T # bes2t
    bcons[0:BC, 19:51] = np.tile(b5e, (BC, 1))                                 # b5rep
    bcons[0:BC, 51:563] = np.tile(np.asarray(inputs["bif"], np.float32), (BC, 1))
    bcons[0:BC, 563:1091] = np.tile(np.asarray(inputs["bdf"], np.float32), (BC, 1))
    bcons[0:BC, 1091:1219] = np.tile(W2f.reshape(128), (BC, 1))                # w2rep
    bcons[0:BC, 1219:1223] = np.tile(np.repeat(np.asarray(inputs["b2f"], np.float32), 2), (BC, 1))
    sh["bcons"] = bcons

    # drop pieces that ride the consolidated streams
    for k in ("w2a", "w2ao", "w2b", "w3", "w4", "w5", "wi2a", "wi2ao", "wi2b"):
        del sh[k]

    # per-core tensors
    obsT = np.ascontiguousarray(obs.reshape(B, 16384).T)  # (16384, 256)
    wes1 = np.asarray(inputs["Wes1"], np.float32)          # (16384, 1024)
    # im2col rows reordered pair-minor: (B/2, 9, 2, 1764), [m, q, j] = sample
    # 2m+j's tap-q row — makes the chunk DMA fully linear on both sides.
    img9 = _im2col9(_pad_flat(patch[:, :1])).reshape(B // 2, 2, 9, 1764).transpose(0, 2, 1, 3)
    ih9 = _im2col9(_pad_flat(patch[:, 1:])).reshape(B // 2, 2, 9, 1764).transpose(0, 2, 1, 3)
    per_core = []
    for c in range(NCORES):
        m = dict(sh)
        m["obsT"] = obsT[c * KSLICE:(c + 1) * KSLICE].astype(bf)
        m["wes1"] = wes1[c * KSLICE:(c + 1) * KSLICE].astype(bf)
        sl = slice(c * BC // 2, (c + 1) * BC // 2)
        m["imgp"] = np.ascontiguousarray(img9[sl]).astype(bf)
        m["ihp"] = np.ascontiguousarray(ih9[sl]).astype(bf)
        per_core.append(m)
    return per_core


# ---------------------------------------------------------------- bass build
def build(debug=(), reps=1, sim=False):
    nc = bacc.Bacc("TRN2", target_bir_lowering=False, debug=False, num_devices=NCORES)

    D = {}

    def din(name, shape, dt=bf16):
        D[name] = nc.dram_tensor(name, list(shape), dt, kind="ExternalInput")
        return D[name]

    obsT_d = din("obsT", (KSLICE, 256))
    wes1_d = din("wes1", (KSLICE, 1024))
    imgp_d = din("imgp", (BC // 2, 9, 2, 1764))
    ihp_d = din("ihp", (BC // 2, 9, 2, 1764))
    w1_d = din("w1", (128, 64))
    wi1_d = din("wi1", (128, 32))
    wbig_d = din("wbig", (128, 13344))
    wsml_d = din("wsml", (64, 320))
    wifp2_d = din("wifp2", (128, 50, 512))
    bcons_d = din("bcons", (128, 1223), f32)

    cc_ins = [nc.dram_tensor(f"cc_in{r}", [256, 1024], bf16) for r in range(reps)]
    cc_outs = [nc.dram_tensor(f"cc_out{r}", [BC, 1024], bf16) for r in range(reps)]
    out_d = nc.dram_tensor("out", [BC, 4], f32, kind="ExternalOutput")

    dbg_handles = {}

    def dbg(name, shape, dt):
        dbg_handles[name] = nc.dram_tensor(name, list(shape), dt, kind="ExternalOutput")
        return dbg_handles[name]

    with tile.TileContext(nc) as tc:
        with tc.tile_pool(name="pw", bufs=1) as pw, \
             tc.tile_pool(name="psum", bufs=3, space="PSUM") as psp:

            _sc = [None]

            def mark(name):
                if _sc[0] is not None:
                    nc.leave_named_scope(_sc[0][0], _sc[0][1], False)
                    _sc[0] = None
                if name:
                    sid, _ = nc.enter_named_scope(name, False)
                    _sc[0] = (name, sid)

            # ---------- persistent weight tiles
            mark("wload")
            def ld(name, shape, src_ap, dt=bf16, pool=None, eng=None):
                t = (pool or pw).tile(list(shape), dt, tag=name)
                (eng or nc.sync).dma_start(out=t[:], in_=src_ap)
                return t

            w1_t = ld("w1", (128, 64), w1_d[:])
            wi1_t = ld("wi1", (128, 32), wi1_d[:])
            bcons_t = ld("bcons", (128, 1223), bcons_d[:], dt=f32)
            bias_t = {
                "bc1": bcons_t[:, 0:1], "bi1c": bcons_t[:, 1:2],
                "bc2": bcons_t[:, 2:3], "bc3": bcons_t[:, 3:5],
                "bc4": bcons_t[0:64, 5:6], "bi2c": bcons_t[:, 6:7],
                "bes1t": bcons_t[:, 7:15], "bes2t": bcons_t[:, 15:19],
                "b5rep": bcons_t[0:BC, 19:51],
                "bifrep": bcons_t[0:BC, 51:563],
                "bdfrep": bcons_t[0:BC, 563:1091],
                "w2rep": bcons_t[0:BC, 1091:1219].rearrange("p (a b) -> p a b", b=32),
                "b2frep": bcons_t[0:BC, 1219:1223],
            }

            def load_deferred_weights(wbig, wsml):
                # one consolidated weight DMA per partition width, split into
                # a few pieces so the stream can interleave on the DMA units;
                # rides the Activation-engine HWDGE ring. Each piece carries a
                # WAW hazard against a gate snippet written once the es1
                # weight stream is mostly in, so these can't starve the front.
                se = nc.scalar
                se.dma_start(out=wbig[:, 12928:13344], in_=wbig_d[:, 12928:13344])
                se.dma_start(out=wbig[:, 0:1152], in_=wbig_d[:, 0:1152])
                se.dma_start(out=wbig[:, 1152:4608], in_=wbig_d[:, 1152:4608])
                se.dma_start(out=wbig[:, 4608:8704], in_=wbig_d[:, 4608:8704])
                se.dma_start(out=wbig[:, 8704:12928], in_=wbig_d[:, 8704:12928])
                se.dma_start(out=wsml[:], in_=wsml_d[:])
                r3 = lambda a, o: a.rearrange("p (d o) -> p d o", o=o)
                global_w = {
                    "w2a": r3(wbig[:, 0:384], 128),
                    "w2ao": r3(wbig[:, 384:768], 128),
                    "w2b": r3(wbig[:, 768:1152], 128),
                    "w3": r3(wbig[:, 1152:3456], 128),
                    "w4": r3(wbig[:, 3456:4608], 64),
                    "wes2": r3(wbig[:, 4608:8704], 512),
                    "wdf": r3(wbig[:, 8704:12928], 528),
                    "wi2a": r3(wbig[:, 12928:13120], 64),
                    "wi2b": r3(wbig[:, 13120:13312], 64),
                    "w5": r3(wsml[:, 0:288], 32),
                    "ident": wsml[0:32, 288:320],
                    "sel": wbig[:, 13312:13344],
                }
                return global_w

            for rep in range(reps):
                # ================= conv stage pools ============================
                wbig_t = pw.tile([128, 13344], bf16, tag="wbig")
                wsml_t = pw.tile([64, 320], bf16, tag="wsml")
                # pcf holds front-only tiles (in-hand conv stage) and is closed
                # right after ihc2 so pwif can reuse its space.
                with tc.tile_pool(name="pconv", bufs=1) as pc:
                    pcf_cm = tc.tile_pool(name="pcf", bufs=1)
                    pcf = pcf_cm.__enter__()
                    # merged parity layout: E samples base rows at partitions
                    # 0:32 (xih) / 0:64 (x1p), dy-shifted copy right above;
                    # O samples at 64:96 / 64:128 with shifted copy above.
                    xih = pcf.tile([128, 16, 22, 22], bf16, tag="xih")
                    hst = pcf.tile([128, 16, 10, 10], bf16, tag="hst")
                    x1pE = pc.tile([128, 16, 22, 22], bf16, tag="x1pE")
                    x1pO = pc.tile([128, 16, 22, 22], bf16, tag="x1pO")
                    h_lin2 = pc.tile([128, BC, 50], bf16, tag="h_lin2")
                    x2 = pc.tile([128, BC, 10, 10], bf16, tag="x2")
                    x3 = pc.tile([128, 2, BC, 8, 8], bf16, tag="x3")
                    x4 = pc.tile([64, BC, 3, 3], bf16, tag="x4")

                    # border zeroing (interiors written by conv evictions)
                    for t_, p0, p1_ in ((xih, 0, 32), (xih, 64, 96), (x1pE, 0, 64), (x1pO, 64, 128)):
                        nc.gpsimd.memset(t_[p0:p1_, :, 0:1, :], 0.0)
                        nc.gpsimd.memset(t_[p0:p1_, :, 21:22, :], 0.0)
                        nc.gpsimd.memset(t_[p0:p1_, :, :, 0:1], 0.0)
                        nc.gpsimd.memset(t_[p0:p1_, :, :, 21:22], 0.0)

                    # ================= ES1: K-split streaming matmul issued
                    # interleaved with ihc1 so the PE queue alternates between
                    # DMA-gated es1 steps and conv work, and the ReduceScatter
                    # fires early enough to hide behind the conv stack.
                    PP2 = 2 * 1764  # t9 per-partition pitch (elems)

                    def imcol_dma(dram9, t9, c0):
                        # 4 linear DMAs per 8-sample chunk: host-expanded
                        # pair-minor im2col rows land at partitions 32*sp + q
                        # (sp=sample-pair base, q=tap), pair on the free dim.
                        for sp in range(4):
                            m = c0 // 2 + sp
                            nc.sync.dma_start(out=t9[32 * sp:32 * sp + 9],
                                              in_=dram9[m, :, :, :])

                    def t9v(t9, sp, j, ys, ny, yst, xs, nx, xst):
                        # moving-operand view: 9 taps at partitions 32sp..32sp+9,
                        # sample parity j, y/x window with strides
                        a = t9[32 * sp:32 * sp + 9].copy()
                        a.ap = bass_rust.VecI64Pair([[PP2, 9], [42 * yst, ny], [xst, nx]])
                        a.offset = 32 * sp * PP2 + j * 1764 + ys * 42 + xs
                        return a

                    with tc.tile_pool(name="pes", bufs=1) as pes, \
                         tc.tile_pool(name="pesp", bufs=2, space="PSUM") as pesp, \
                         tc.tile_pool(name="pim", bufs=3) as pim, \
                         tc.tile_pool(name="pev", bufs=4) as pev:

                        mark("es1")
                        obsT_t = ld("obsT", (128, 16, 256),
                                    obsT_d[:].rearrange("(t p) b -> p t b", p=128),
                                    pool=pes)
                        # wes1 slice fully SBUF-resident: rounds 1-3 run with
                        # zero DMA dependence. First half issued now; second
                        # half after the first conv im2col DMAs so those don't
                        # queue behind the whole stream.
                        wes1r = pes.tile([128, 16, 1024], bf16, tag="wes1r")

                        def wes1_load(c0, c1_):
                            for cch in range(c0, c1_):
                                nc.sync.dma_start(
                                    out=wes1r[:, 2 * cch:2 * cch + 2, :],
                                    in_=wes1_d[256 * cch:256 * (cch + 1), :]
                                    .rearrange("(t p) o -> p t o", p=128))

                        wes1_load(0, 4)
                        es1s = pes.tile([128, 2, 2, 512], bf16, tag="es1s")

                        def es1_round(r):
                            mark("es1")
                            if r == 0:
                                wes1_load(4, 8)
                            bb, nt = divmod(r, 2)
                            accr = pesp.tile([128, 512], f32, name="accr", tag="es1p")
                            for kt in range(16):
                                nc.tensor.matmul(accr[:],
                                                 obsT_t[:, kt, bb * 128:(bb + 1) * 128],
                                                 wes1r[:, kt, nt * 512:(nt + 1) * 512],
                                                 start=(kt == 0), stop=(kt == 15))
                            nc.vector.tensor_copy(es1s[:, bb, nt, :], accr[:])
                            if r == 0:
                                # gate release: wes1 is fully resident once
                                # round 0 completes — poke each deferred
                                # weight region (WAW hazard holds them back).
                                for col in (1151, 4607, 8703, 12927, 13343):
                                    nc.vector.tensor_copy(wbig_t[0:1, col:col + 1], es1s[0:1, 0, 0, 0:1])
                                nc.vector.tensor_copy(wsml_t[0:1, 319:320], es1s[0:1, 0, 0, 0:1])

                        # ---------- ihc1 chunks interleaved with es1 steps
                        # 8 concurrent tiles: 4 sample bases (rows) x E/O (cols)
                        def ihc1_chunk(chunk):
                            mark("ihc1")
                            t9 = pim.tile([128, 2, 42, 42], bf16, tag="t9")
                            imcol_dma(ihp_d, t9, chunk * 8)
                            for sp in range(4):
                                m = chunk * 4 + sp
                                pp = psp.tile([128, 2, 512], f32, tag="mm")
                                ppy = pp[:, 0, 0:400].rearrange("p (y x) -> p y x", x=20)
                                nc.tensor.matmul(ppy[0:32], wi1_t[32 * sp:32 * sp + 9, :],
                                                 t9v(t9, sp, 0, 0, 20, 2, 0, 20, 2),
                                                 start=True, stop=True, tile_position=(32 * sp, 0))
                                nc.tensor.matmul(ppy[64:96], wi1_t[32 * sp:32 * sp + 9, :],
                                                 t9v(t9, sp, 1, 0, 20, 2, 0, 20, 2),
                                                 start=True, stop=True, tile_position=(32 * sp, 64))
                                nc.scalar.activation(xih[0:32, m, 1:21, 1:21], ppy[0:32],
                                                     AF.Relu, bias=bias_t["bi1c"][0:32, 0:1])
                                nc.vector.tensor_scalar(xih[64:96, m, 1:21, 1:21], ppy[64:96],
                                                        bias_t["bi1c"][64:96, 0:1], 0.0, ALU.add, ALU.max)

                        # ---------- c1 chunk: 4 concurrent tiles (2 bases x
                        # E/O), fused pool
                        def c1_chunk(chunk):
                            mark("c1")
                            t9 = pim.tile([128, 2, 42, 42], bf16, name="t9", tag="t9")
                            imcol_dma(imgp_d, t9, chunk * 8)
                            for qp in range(2):  # quarter-pairs: rows 20*qp..20*qp+20
                                for sp in range(4):
                                    m = chunk * 4 + sp
                                    pp = psp.tile([128, 2, 512], f32, name="pp", tag="mm")
                                    for q in range(2):
                                        for j in range(2):
                                            nc.tensor.matmul(
                                                pp[64 * j:64 * (j + 1), q, 0:400]
                                                .rearrange("p (y x) -> p y x", x=40),
                                                w1_t[32 * sp:32 * sp + 9, :],
                                                t9v(t9, sp, j, qp * 20 + q * 10, 10, 1, 0, 40, 1),
                                                start=True, stop=True, tile_position=(32 * sp, 64 * j))
                                    # evict+relu+bias on Act; x-max + y-max on DVE
                                    ppv = pp[:, :, 0:400].rearrange("p q (y x) -> p q y x", x=40)
                                    tt = pev.tile([128, 2, 10, 40], bf16, name="tt", tag="c1e")
                                    nc.scalar.activation(tt[:], ppv, AF.Relu, bias=bias_t["bc1"][:, 0:1])
                                    hh = pev.tile([128, 2, 10, 20], bf16, name="hh", tag="c1h")
                                    nc.vector.tensor_tensor(hh[:], tt[:, :, :, 0:40:2], tt[:, :, :, 1:40:2], ALU.max)
                                    r0 = 1 + qp * 10
                                    nc.vector.tensor_tensor(
                                        x1pE[0:64, m, r0:r0 + 10, 1:21]
                                        .rearrange("p (q y) x -> p q y x", q=2),
                                        hh[0:64, :, 0:10:2, :], hh[0:64, :, 1:10:2, :], ALU.max)
                                    nc.vector.tensor_tensor(
                                        x1pO[64:128, m, r0:r0 + 10, 1:21]
                                        .rearrange("p (q y) x -> p q y x", q=2),
                                        hh[64:128, :, 0:10:2, :], hh[64:128, :, 1:10:2, :], ALU.max)
                            # per-chunk dy-shift copies unblock c2 sample-wise
                            c0 = chunk * 4
                            nc.sync.dma_start(out=x1pE[64:128, c0:c0 + 4, 0:21, :],
                                              in_=x1pE[0:64, c0:c0 + 4, 1:22, :])
                            nc.sync.dma_start(out=x1pO[0:64, c0:c0 + 4, 0:21, :],
                                              in_=x1pO[64:128, c0:c0 + 4, 1:22, :])

                        for i in range(4):
                            ihc1_chunk(i)
                            c1_chunk(i)
                            es1_round(i)
                        nc.sync.dma_start(out=xih[32:64, :, 0:21, :], in_=xih[0:32, :, 1:22, :])
                        nc.sync.dma_start(out=xih[96:128, :, 0:21, :], in_=xih[64:96, :, 1:22, :])

                        mark("es1")
                        nc.sync.dma_start(out=cc_ins[rep][:].rearrange("(bb p) (nt o) -> p bb nt o", p=128, o=512),
                                          in_=es1s[:])
                        if sim:
                            nc.sync.dma_start(out=cc_outs[rep][:], in_=cc_ins[rep][0:BC, :])
                        else:
                            nc.gpsimd.collective_compute(
                                "ReduceScatter", ALU.add, replica_groups=[list(range(NCORES))],
                                ins=[cc_ins[rep][:]], outs=[cc_outs[rep][:]])
                        esb = pw.tile([BC, 1024], bf16, tag="esb")
                        nc.gpsimd.dma_start(out=esb[:], in_=cc_outs[rep][:])
                        if "dbg_es1" in debug:
                            nc.sync.dma_start(out=dbg("dbg_es1", (BC, 1024), bf16)[:], in_=esb[:])

                    gw = load_deferred_weights(wbig_t, wsml_t)
                    w2a_t, w2ao_t, w2b_t = gw["w2a"], gw["w2ao"], gw["w2b"]
                    w3_t, w4_t, w5_t = gw["w3"], gw["w4"], gw["w5"]
                    wi2a_t, wi2b_t = gw["wi2a"], gw["wi2b"]
                    wes2_t, wdf_t, ident_t = gw["wes2"], gw["wdf"], gw["ident"]

                    # ---------- ihc2: dy-stacked stride-2 conv, 4 pair-slots
                    # per matmul; E rows 0:64 and O rows 64:128 run on disjoint
                    # (row,col) subarray quadrants.
                    mark("ihc2")
                    for g in range(4):
                        pp = psp.tile([128, 2, 512], f32, tag="mm")
                        ppv = pp[:, 0, 0:400].rearrange("p (s y x) -> p s y x", y=10, x=10)
                        for dx in range(3):
                            nc.tensor.matmul(ppv[0:64], wi2a_t[0:64, dx, :],
                                             xih[0:64, 4 * g:4 * g + 4, 0:20:2, dx:dx + 20:2],
                                             start=(dx == 0), stop=False, tile_position=(0, 0))
                        for dx in range(3):
                            nc.tensor.matmul(ppv[0:64], wi2b_t[32:64, dx, :],
                                             xih[32:64, 4 * g:4 * g + 4, 1:21:2, dx:dx + 20:2],
                                             start=False, stop=(dx == 2), tile_position=(32, 0))
                        for dx in range(3):
                            nc.tensor.matmul(ppv[64:128], wi2a_t[64:128, dx, :],
                                             xih[64:128, 4 * g:4 * g + 4, 0:20:2, dx:dx + 20:2],
                                             start=(dx == 0), stop=False, tile_position=(64, 64))
                        for dx in range(3):
                            nc.tensor.matmul(ppv[64:128], wi2b_t[96:128, dx, :],
                                             xih[96:128, 4 * g:4 * g + 4, 1:21:2, dx:dx + 20:2],
                                             start=False, stop=(dx == 2), tile_position=(96, 64))
                        nc.scalar.activation(hst[0:64, 4 * g:4 * g + 4, :, :], ppv[0:64],
                                             AF.Relu, bias=bias_t["bi2c"][0:64, 0:1])
                        nc.vector.tensor_scalar(hst[64:128, 4 * g:4 * g + 4, :, :], ppv[64:128],
                                                bias_t["bi2c"][64:128, 0:1], 0.0, ALU.add, ALU.max)
                    # h_lin2[(pixgroup, ch), s, q]: rows 0-63 = pix q, 64-127 = pix q+50
                    nc.sync.dma_start(out=h_lin2[0:64, 0:32:2, :],
                                      in_=hst[0:64, :, 0:5, :].rearrange("p k a b -> p k (a b)"))
                    nc.sync.dma_start(out=h_lin2[0:64, 1:32:2, :],
                                      in_=hst[64:128, :, 0:5, :].rearrange("p k a b -> p k (a b)"))
                    nc.sync.dma_start(out=h_lin2[64:128, 0:32:2, :],
                                      in_=hst[0:64, :, 5:10, :].rearrange("p k a b -> p k (a b)"))
                    nc.sync.dma_start(out=h_lin2[64:128, 1:32:2, :],
                                      in_=hst[64:128, :, 5:10, :].rearrange("p k a b -> p k (a b)"))
                    if "dbg_hlin" in debug:
                        nc.sync.dma_start(out=dbg("dbg_hlin", (128, BC, 50), bf16)[:], in_=h_lin2[:])
                    pcf_cm.__exit__(None, None, None)

                    # ---------- c2: dy-stacked 3x3 conv + pool
                    mark("c2")
                    # pairs issued together: the two dy2 tap sets use disjoint
                    # PE row groups (E rows 64:128, O rows 0:64) and overlap
                    for m in range(16):
                        ppE = psp.tile([128, 20, 20], f32, name="ppE", tag="mm")
                        ppO = psp.tile([128, 20, 20], f32, name="ppO", tag="mm")
                        for dx in range(3):
                            nc.tensor.matmul(ppE[:], w2a_t[:, dx, :], x1pE[:, m, 0:20, dx:dx + 20],
                                             start=(dx == 0), stop=False)
                        for dx in range(3):
                            nc.tensor.matmul(ppO[:], w2ao_t[:, dx, :], x1pO[:, m, 0:20, dx:dx + 20],
                                             start=(dx == 0), stop=False)
                        for dx in range(3):
                            nc.tensor.matmul(ppE[:], w2b_t[64:128, dx], x1pE[64:128, m, 1:21, dx:dx + 20],
                                             start=False, stop=(dx == 2), tile_position=(64, 0))
                        for dx in range(3):
                            nc.tensor.matmul(ppO[:], w2b_t[0:64, dx], x1pO[0:64, m, 1:21, dx:dx + 20],
                                             start=False, stop=(dx == 2), tile_position=(0, 0))
                        for pp, s in ((ppE, 2 * m), (ppO, 2 * m + 1)):
                            t2 = pc.tile([128, 20, 20], bf16, name="t2", tag="c2e")
                            nc.scalar.activation(t2[:], pp[:], AF.Relu, bias=bias_t["bc2"][:, 0:1])
                            h2 = pc.tile([128, 20, 10], bf16, name="h2", tag="c2h")
                            nc.vector.tensor_tensor(h2[:], t2[:, :, 0:20:2], t2[:, :, 1:20:2], ALU.max)
                            nc.vector.tensor_tensor(x2[:, s, :, :], h2[:, 0:20:2, :], h2[:, 1:20:2, :], ALU.max)
                    if "dbg_x2" in debug:
                        nc.sync.dma_start(out=dbg("dbg_x2", (128, BC, 10, 10), bf16)[:], in_=x2[:])

                    with tc.tile_pool(name="psacc", bufs=1, space="PSUM") as psacc, \
                         tc.tile_pool(name="pssm", bufs=1, space="PSUM") as pssm, \
                         tc.tile_pool(name="pwif", bufs=2) as pwif:
                        # ---------- ihv: 50 pix-paired accumulating matmuls (K=128)
                        mark("ihv")
                        # 4 pixel-groups accumulate concurrently on 4 PE col
                        # groups (M=32 each); summed on eviction.
                        p_ihv = psacc.tile([128, 512], f32, tag="ihv")
                        qsplits = [(0, 13), (13, 13), (26, 13), (39, 11)]
                        for q0, qn in qsplits:
                            wifc = pwif.tile([128, 13, 512], bf16, tag="wifc")
                            nc.sync.dma_start(out=wifc[:, 0:qn, :], in_=wifp2_d[:, q0:q0 + qn, :])
                            for q in range(qn):
                                qg = q0 + q
                                j = qg % 4
                                nc.tensor.matmul(p_ihv[32 * j:32 * j + 32, :],
                                                 h_lin2[:, :, qg], wifc[:, q, :],
                                                 start=(qg == j), stop=(qg == 46 + ((j - 2) % 4)),
                                                 tile_position=(0, 32 * j), skip_group_check=True)
                        # cross-colgroup sum via selector matmul (DVE is
                        # partition-locked), then bias+relu
                        ihs = pc.tile([128, 512], bf16, tag="ihs")
                        for j in range(4):
                            nc.vector.tensor_copy(ihs[32 * j:32 * j + 32, :], p_ihv[32 * j:32 * j + 32, :])
                        pihv2 = pssm.tile([BC, 512], f32, name="pihv2", tag="sm")
                        nc.tensor.matmul(pihv2[:], gw["sel"], ihs[:], start=True, stop=True)
                        ihv_f = pc.tile([BC, 512], f32, tag="ihv_f")
                        nc.vector.tensor_tensor(ihv_f[:], bias_t["bifrep"][:], pihv2[:], ALU.add)
                        ihvb = pc.tile([BC, 512], bf16, tag="ihvb")
                        nc.vector.tensor_scalar(ihvb[:], ihv_f[:], 0.0, None, ALU.max)
                        if "dbg_ihv" in debug:
                            nc.sync.dma_start(out=dbg("dbg_ihv", (BC, 512), bf16)[:], in_=ihvb[:])

                        catT = pw.tile([128, 8, BC], bf16, tag="catT")
                        for k in range(4):
                            pt = pssm.tile([128, BC], bf16, tag="sm")
                            nc.tensor.transpose(pt[:], ihvb[:, 128 * k:128 * (k + 1)], ident_t[:])
                            nc.vector.tensor_copy(catT[:, 4 + k, :], pt[:])

                        # ---------- c3
                        mark("c3")
                        for mt in range(2):
                            for sg in range(4):
                                pp = psp.tile([128, 8, 8, 8], f32, tag="mm")
                                first = True
                                for dy in range(3):
                                    for dx in range(3):
                                        nc.tensor.matmul(pp[:], w3_t[:, mt * 9 + dy * 3 + dx, :],
                                                         x2[:, sg * 8:(sg + 1) * 8, dy:dy + 8, dx:dx + 8],
                                                         start=first, stop=(dy == 2 and dx == 2))
                                        first = False
                                nc.scalar.activation(x3[:, mt, sg * 8:(sg + 1) * 8, :, :], pp[:],
                                                     AF.Relu, bias=bias_t["bc3"][:, mt:mt + 1])
                        if "dbg_x3" in debug:
                            nc.sync.dma_start(out=dbg("dbg_x3", (128, 2, BC, 8, 8), bf16)[:], in_=x3[:])

                        # ---------- c4 + pool
                        mark("c4")
                        for sg in range(4):
                            pp = psp.tile([64, 8, 6, 6], f32, tag="mm")
                            first = True
                            for kt in range(2):
                                for dy in range(3):
                                    for dx in range(3):
                                        nc.tensor.matmul(pp[:], w4_t[:, kt * 9 + dy * 3 + dx, :],
                                                         x3[:, kt, sg * 8:(sg + 1) * 8, dy:dy + 6, dx:dx + 6],
                                                         start=first, stop=(kt == 1 and dy == 2 and dx == 2))
                                        first = False
                            t4 = pc.tile([64, 8, 6, 6], bf16, tag="c4e")
                            nc.scalar.activation(t4[:], pp[:], AF.Relu, bias=bias_t["bc4"][:, 0:1])
                            h4 = pc.tile([64, 8, 6, 3], bf16, tag="c4h")
                            nc.vector.tensor_tensor(h4[:], t4[:, :, :, 0:6:2], t4[:, :, :, 1:6:2], ALU.max)
                            nc.vector.tensor_tensor(x4[:, sg * 8:(sg + 1) * 8, :, :],
                                                    h4[:, :, 0:6:2, :], h4[:, :, 1:6:2, :], ALU.max)

                        # ---------- c5 (batch-major out: samples on partitions)
                        mark("c5")
                        pp5 = pssm.tile([BC, 32], f32, tag="sm")
                        for q in range(9):
                            dy, dx = divmod(q, 3)
                            nc.tensor.matmul(pp5[:], x4[:, :, dy, dx], w5_t[:, q, :],
                                             start=(q == 0), stop=(q == 8))
                        xs_t = pc.tile([BC, 16, 2], f32, tag="xs")
                        xs_p = pc.tile([BC, 16, 2], f32, tag="xs_p")
                        nc.vector.tensor_tensor(xs_p[:], pp5[:].rearrange("p (a b) -> p a b", b=2),
                                                bias_t["b5rep"][:].rearrange("p (a b) -> p a b", b=2), ALU.add)
                        nc.vector.tensor_scalar(xs_t[:], xs_p[:], 0.0, None, ALU.max)
                        xg1 = pc.tile([BC, 16, 2], f32, tag="xg1")
                        nc.vector.tensor_copy(xg1[:], xs_t[:, :, ::-1])
                        if "dbg_xf" in debug:
                            nc.sync.dma_start(out=dbg("dbg_xf", (BC, 16, 2), f32)[:], in_=xs_t[:])

                        # ---------- es tail: transpose RS output, bias+relu, es2
                        mark("estail")
                        esT = pw.tile([128, 8, BC], bf16, tag="esT")
                        for t in range(8):
                            pt = pssm.tile([128, BC], bf16, tag="sm")
                            nc.tensor.transpose(pt[:], esb[:, 128 * t:128 * (t + 1)], ident_t[:])
                            nc.vector.tensor_scalar(esT[:, t, :], pt[:], bias_t["bes1t"][:, t:t + 1],
                                                    0.0, ALU.add, ALU.max)
                        if "dbg_esT" in debug:
                            nc.sync.dma_start(out=dbg("dbg_esT", (128, 8, BC), bf16)[:], in_=esT[:])
                        for mt in range(4):
                            pp = pssm.tile([128, BC], f32, tag="sm")
                            for t in range(8):
                                nc.tensor.matmul(pp[:], wes2_t[:, t, mt * 128:(mt + 1) * 128], esT[:, t, :],
                                                 start=(t == 0), stop=(t == 7))
                            nc.vector.tensor_scalar(catT[:, mt, :], pp[:], bias_t["bes2t"][:, mt:mt + 1],
                                                    0.0, ALU.add, ALU.max)
                        if "dbg_catT" in debug:
                            nc.sync.dma_start(out=dbg("dbg_catT", (128, 8, BC), bf16)[:], in_=catT[:])

                        # ---------- df: dynamic filter weights (batch-major psum)
                        mark("df")
                        pdf1 = pssm.tile([BC, 512], f32, tag="sm")
                        pdf2 = psacc.tile([BC, 16], f32, tag="ihv")
                        for t in range(8):
                            nc.tensor.matmul(pdf1[:], catT[:, t, :], wdf_t[:, t, 0:512],
                                             start=(t == 0), stop=(t == 7))
                        for t in range(8):
                            nc.tensor.matmul(pdf2[:], catT[:, t, :], wdf_t[:, t, 512:528],
                                             start=(t == 0), stop=(t == 7))
                        wb_sb = pc.tile([BC, 528], f32, tag="wb_sb")
                        nc.vector.tensor_tensor(wb_sb[:, 0:512], pdf1[:], bias_t["bdfrep"][:, 0:512], ALU.add)
                        nc.vector.tensor_tensor(wb_sb[:, 512:528], pdf2[:], bias_t["bdfrep"][:, 512:528], ALU.add)
                        if "dbg_wb" in debug:
                            nc.sync.dma_start(out=dbg("dbg_wb", (BC, 528), f32)[:], in_=wb_sb[:])

                        # ---------- dynamic 1x1 group conv tail (all DVE)
                        mark("tail")
                        wbv = wb_sb[:, 0:512].rearrange("p (o j) -> p o j", j=32)
                        tmp0 = pc.tile([BC, 16, 32], f32, tag="tmp0")
                        tmp1 = pc.tile([BC, 16, 32], f32, tag="tmp1")
                        xb0 = xs_t[:].rearrange("p a b -> p (a b)").unsqueeze(1).broadcast_to((BC, 16, 32))
                        xb1 = xg1[:].rearrange("p a b -> p (a b)").unsqueeze(1).broadcast_to((BC, 16, 32))
                        nc.vector.tensor_mul(tmp0[:], wbv, xb0)
                        nc.vector.tensor_mul(tmp1[:], wbv, xb1)
                        featr = pc.tile([BC, 16, 2], f32, tag="featr")
                        f0 = pc.tile([BC, 16], f32, tag="f0")
                        f1 = pc.tile([BC, 16], f32, tag="f1")
                        nc.vector.tensor_reduce(f0[:], tmp0[:], mybir.AxisListType.X, ALU.add)
                        nc.vector.tensor_reduce(f1[:], tmp1[:], mybir.AxisListType.X, ALU.add)
                        nc.vector.tensor_tensor(featr[:, :, 0], f0[:], wb_sb[:, 512:528], ALU.add)
                        nc.vector.tensor_tensor(featr[:, :, 1], f1[:], wb_sb[:, 512:528], ALU.add)
                        nc.vector.tensor_scalar(featr[:], featr[:], 0.0, None, ALU.max)
                        fb_ = featr[:].rearrange("p a b -> p (a b)").unsqueeze(1).broadcast_to((BC, 4, 32))
                        tmp2 = pc.tile([BC, 4, 32], f32, tag="tmp2")
                        nc.vector.tensor_mul(tmp2[:], bias_t["w2rep"][:], fb_)
                        o4_t = pc.tile([BC, 4], f32, tag="o4")
                        nc.vector.tensor_reduce(o4_t[:], tmp2[:], mybir.AxisListType.X, ALU.add)
                        outsb = pc.tile([BC, 4], f32, tag="outsb")
                        nc.vector.tensor_tensor(outsb[:], o4_t[:], bias_t["b2frep"][:], ALU.add)
                        nc.sync.dma_start(out=out_d[:], in_=outsb[:])
                        mark(None)

    nc.compile()
    return nc, dbg_handles


# ---------------------------------------------------------------- run
_CACHE = {}


def _get_module(debug=(), reps=1, sim=False):
    key = (tuple(sorted(debug)), reps, sim)
    if key not in _CACHE:
        _CACHE[key] = build(debug, reps, sim)
    return _CACHE[key]


def run(inputs, debug=()):
    nc, dbg_handles = _get_module(debug)
    in_maps = host_prep(inputs)
    res = run_bass_kernel_spmd(nc, in_maps, list(range(NCORES)))
    return res


def kernel(**inputs):
    res = run(inputs)
    out = np.concatenate([np.asarray(res.results[c]["out"], np.float32) for c in range(NCORES)], axis=0)
    return out.reshape(256, 2, 2)


# ---------------------------------------------------------------- timing
def make_runner(nc, in_maps):
    """Builds a reusable jitted executor for `nc` (mirrors
    bass2jax.run_bass_via_pjrt's multi-core path) with device-resident inputs.
    Returns a zero-arg callable that executes once and blocks."""
    import jax
    import numpy as _np
    from jax.sharding import Mesh, PartitionSpec
    from jax.experimental.shard_map import shard_map
    from concourse import bass2jax as b2j

    b2j.install_neuronx_cc_hook()
    n_cores = len(in_maps)
    partition_name = nc.partition_id_tensor.name if nc.partition_id_tensor else None
    in_names, out_names, out_avals, zero_outs = [], [], [], []
    for alloc in nc.m.functions[0].allocations:
        if not isinstance(alloc, mybir.MemoryLocationSet):
            continue
        name = alloc.memorylocations[0].name
        if alloc.kind == "ExternalInput":
            if name != partition_name:
                in_names.append(name)
        elif alloc.kind == "ExternalOutput":
            out_names.append(name)
            shape = tuple(alloc.tensor_shape)
            dtype = mybir.dt.np(alloc.dtype)
            out_avals.append(jax.core.ShapedArray(shape, dtype))
            zero_outs.append(_np.zeros(shape, dtype))
    n_params = len(in_names)
    n_outs = len(out_avals)
    all_in_names = list(in_names) + out_names
    if partition_name is not None:
        all_in_names.append(partition_name)

    def _body(*args):
        operands = list(args)
        if partition_name is not None:
            operands.append(b2j.partition_id_tensor())
        outs = b2j._bass_exec_p.bind(
            *operands,
            out_avals=tuple(out_avals),
            in_names=tuple(all_in_names),
            out_names=tuple(out_names),
            lowering_input_output_aliases=(),
            sim_require_finite=True,
            sim_require_nnan=True,
            nc=nc,
        )
        return tuple(outs)

    devices = jax.devices()[:n_cores]
    mesh = Mesh(_np.asarray(devices), ("core",))
    in_specs = (PartitionSpec("core"),) * (n_params + n_outs)
    out_specs = (PartitionSpec("core"),) * len(out_names)
    donate = tuple(range(n_params, n_params + n_outs))
    sharded = jax.jit(
        shard_map(_body, mesh=mesh, in_specs=in_specs, out_specs=out_specs,
                  check_rep=False),
        donate_argnums=donate, keep_unused=True)
    concat_in = [
        _np.concatenate([_np.asarray(in_maps[c][nm]) for c in range(n_cores)], axis=0)
        for nm in in_names
    ]
    from jax.sharding import NamedSharding
    shard = NamedSharding(mesh, PartitionSpec("core"))
    in_dev = [jax.device_put(a, shard) for a in concat_in]
    zshapes = [((n_cores * z.shape[0],) + z.shape[1:], z.dtype) for z in zero_outs]

    def call():
        zs = [jax.device_put(_np.zeros(s, d), shard) for s, d in zshapes]
        outs = sharded(*in_dev, *zs)
        jax.block_until_ready(outs)
        return outs

    return call


def time_kernel_reps(inputs, iters=8, reps=4):
    """Differential in-program repetition timing: builds reps=1 and reps=N
    modules, times both through the same dispatch path, and attributes
    (tN - t1)/(N-1) to one kernel execution."""
    import time
    in_maps = host_prep(inputs)
    best = {}
    for r in (1, reps):
        nc, _ = _get_module((), r)
        call = make_runner(nc, in_maps)
        call()
        ts = []
        for _ in range(iters):
            t0 = time.perf_counter()
            call()
            ts.append(time.perf_counter() - t0)
        ts.sort()
        best[r] = ts[:3]
    import numpy as _np
    t1 = _np.mean(best[1])
    tN = _np.mean(best[reps])
    return (tN - t1) / (reps - 1) * 1e9, t1 * 1e9, tN * 1e9


def time_kernel(inputs, iters=10):
    """Returns (best_ns, floor_ns): wall time of one kernel execution and of a
    null kernel through the same dispatch path."""
    import time
    nc, _ = _get_module(())
    in_maps = host_prep(inputs)
    call = make_runner(nc, in_maps)
    call()
    ts = []
    for _ in range(iters):
        t0 = time.perf_counter()
        call()
        ts.append(time.perf_counter() - t0)
    best = min(ts)

    # null kernel floor
    key = "_null"
    if key not in _CACHE:
        ncn = bacc.Bacc("TRN2", target_bir_lowering=False, debug=False, num_devices=NCORES)
        xi = ncn.dram_tensor("x", [128, 4], f32, kind="ExternalInput")
        yo = ncn.dram_tensor("y", [128, 4], f32, kind="ExternalOutput")
        with tile.TileContext(ncn) as tcn:
            with tcn.tile_pool(name="p", bufs=1) as pool:
                t = pool.tile([128, 4], f32)
                ncn.sync.dma_start(out=t[:], in_=xi[:])
                ncn.sync.dma_start(out=yo[:], in_=t[:])
        ncn.compile()
        _CACHE[key] = ncn
    ncn = _CACHE[key]
    calln = make_runner(ncn, [{"x": np.zeros((128, 4), np.float32)}] * NCORES)
    calln()
    tn = []
    for _ in range(iters):
        t0 = time.perf_counter()
        calln()
        tn.append(time.perf_counter() - t0)
    floor = min(tn)
    return best * 1e9, floor * 1e9

